# revision 4
# baseline (speedup 1.0000x reference)
"""Grouped Conv2d (512 groups, 2->2 ch/group, 3x3 VALID) on 8 trn2 NeuronCores.

Strategy (hybrid, fp16 data path):
  - Shard the 512 groups across 8 cores: 64 groups = 128 channels per core.
    Fully independent (no collectives); batch stays whole on every core.
  - Row-split each batch's 54 output rows across engines:
      * PE rows [0, R_PE): block-diagonal 128x128 weight per 3x3 tap; 9
        accumulating fp16 matmuls per PSUM chunk (<=9 rows x 54 cols); ACT
        evicts PSUM -> fp16 SBUF (interleaved with its product ops).
      * DVE rows: per-term tensor_scalar product (4x mode) into td, then
        tensor_tensor accumulate (2x mode); 18 terms = 9 taps x {diag,
        pair-swapped}.
      * Pool rows: accumulated on gpsimd from td tail rows + ACT products.
    Per-group 2x2 channel mixing needs x[p^1] at partition p: host supplies
    a channel pair-swapped copy `xs` of the bottom x rows.
  - fp16 halves DMA bytes vs fp32 (x converted on host; y back on host).
"""

import sys

import numpy as np

for _p in ("/opt/trn_rl_repo",):
    if _p not in sys.path:
        sys.path.insert(0, _p)

import concourse.bacc as bacc
import concourse.bass as bass
import concourse.tile as tile
from concourse import mybir
from concourse.bass_utils import run_bass_kernel_spmd

N_CORES = 8
B, C, H, W = 16, 1024, 56, 56
KH = KW = 3
HO, WO = H - KH + 1, W - KW + 1  # 54, 54
CPC = C // N_CORES  # 128 channels (64 groups) per core

R_PE = 43   # rows computed on the PE (psum chunks of <=9 rows)
R_DVE = 8   # rows accumulated on DVE
R_PTD = 1   # rows accumulated on Pool from DVE-produced td tail
R_ACT = HO - R_PE - R_DVE - R_PTD  # rows produced by ACT, added on Pool
VSTART = R_PE
R_TD = R_DVE + R_PTD  # td product rows (one DVE tensor_scalar per term)
R_V = HO - R_PE
XS_ROWS = R_V + KH - 1  # x rows needed by the vector region

# PSUM chunks for the PE region
_CHUNKS = []
_r = 0
while _r < R_PE:
    _CHUNKS.append((_r, min(9, R_PE - _r)))
    _r += min(9, R_PE - _r)

# ACT-stream interleave: number of vector terms emitted before each chunk's
# eviction (so evictions land when their chunk finishes, products fill gaps).
_TERMS_BEFORE_EVICT = [7, 5, 5, 1, 0]
assert sum(_TERMS_BEFORE_EVICT) <= 2 * KH * KW
assert len(_TERMS_BEFORE_EVICT) == len(_CHUNKS)

_NC_CACHE = {}


def _build_program():
    nc = bacc.Bacc(
        "TRN2", target_bir_lowering=False, debug=False, num_devices=N_CORES
    )
    f32 = mybir.dt.float32
    f16 = mybir.dt.float16

    x_d = nc.declare_dram_parameter("x", [B, CPC, H, W], f16, isOutput=False)
    xs_d = nc.declare_dram_parameter(
        "xs", [B, CPC, XS_ROWS, W], f16, isOutput=False
    )
    wm_d = nc.declare_dram_parameter(
        "wm", [CPC, KH * KW, CPC], f16, isOutput=False
    )
    wv_d = nc.declare_dram_parameter("wv", [CPC, 2, KH * KW], f32, isOutput=False)
    y_d = nc.declare_dram_parameter("y", [B, CPC, HO, WO], f16, isOutput=True)

    with tile.TileContext(nc) as tc:
        with (
            tc.tile_pool(name="wpool", bufs=1) as wpool,
            tc.tile_pool(name="xpool", bufs=3) as xpool,
            tc.tile_pool(name="xspool", bufs=3) as xspool,
            tc.tile_pool(name="oppool", bufs=3) as oppool,
            tc.tile_pool(name="odpool", bufs=3) as odpool,
            tc.tile_pool(name="ovpool", bufs=3) as ovpool,
            tc.tile_pool(name="tdpool", bufs=4) as tdpool,
            tc.tile_pool(name="tmpool", bufs=4) as tmpool,
            tc.tile_pool(name="psum", bufs=7, space="PSUM") as ppool,
            tc.tile_pool(name="scratch", bufs=1, space="PSUM") as spool,
        ):
            wt = wpool.tile([CPC, KH * KW, CPC], f16)
            nc.sync.dma_start(out=wt[:], in_=wm_d[:])
            wvt = wpool.tile([CPC, 2, KH * KW], f32)
            nc.sync.dma_start(out=wvt[:], in_=wv_d[:])

            # The fused matmul (LDW+MM) supports only ONE semaphore wait;
            # these sync matmuls absorb DMA waits so real matmuls only
            # depend on PE program order.
            scr = spool.tile([CPC, 512], f32)
            nc.tensor.matmul(
                scr[:, :2], lhsT=wt[:, 0, :], rhs=wt[:, 0, :2],
                start=True, stop=True,
            )
            # Dummy matmuls keep PE busy during the initial x DMA fill so
            # the HAM clock gate ramps to 2.4 GHz before real work arrives.
            for _ in range(16):
                nc.tensor.matmul(
                    scr[:, :256], lhsT=wt[:, 0, :], rhs=wt[:, 0:2, :],
                    start=True, stop=True,
                )

            pools = (xpool, xspool, oppool, odpool, ovpool, tdpool, tmpool, ppool)
            for n in range(B):
                _emit_batch(nc, pools, x_d, xs_d, y_d, wt, wvt, scr, n)
    nc.compile()
    return nc


def _emit_batch(nc, pools, x_d, xs_d, y_d, wt, wvt, scr, n):
    (xpool, xspool, oppool, odpool, ovpool, tdpool, tmpool, ppool) = pools
    f32 = mybir.dt.float32
    f16 = mybir.dt.float16
    HSPLIT = 30
    OSPLIT = 27
    Copy = mybir.ActivationFunctionType.Copy
    add = mybir.AluOpType.add
    mult = mybir.AluOpType.mult

    xt = xpool.tile([CPC, H, W], f16)
    nc.sync.dma_start(out=xt[:, :HSPLIT, :], in_=x_d[n, :, :HSPLIT, :])
    nc.sync.dma_start(out=xt[:, HSPLIT:, :], in_=x_d[n, :, HSPLIT:, :])
    xst = xspool.tile([CPC, XS_ROWS, W], f16)
    nc.sync.dma_start(out=xst[:], in_=xs_d[n])

    # absorb the two x-DMA semaphores ahead of the real matmuls
    nc.tensor.matmul(
        scr[:, :2], lhsT=wt[:, 0, :], rhs=xt[:, 0, :2], start=True, stop=True
    )
    nc.tensor.matmul(
        scr[:, :2], lhsT=wt[:, 0, :], rhs=xt[:, H - 1, :2],
        start=True, stop=True,
    )

    op = oppool.tile([CPC, R_PE, WO], f16)   # PE-region rows (ACT evicts)
    od = odpool.tile([CPC, R_DVE, WO], f16)  # DVE-accumulated rows
    ov = ovpool.tile([CPC, R_PTD + R_ACT, WO], f16)  # Pool-accumulated rows

    taps = [(kh, kw, j) for kh in range(KH) for kw in range(KW) for j in range(2)]

    def emit_term(i):
        kh, kw, j = taps[i]
        t = kh * KW + kw
        if j == 0:
            dsl = xt[:, VSTART + kh : VSTART + kh + R_TD, kw : kw + WO]
            msl = xt[
                :,
                VSTART + R_TD + kh : VSTART + R_TD + kh + R_ACT,
                kw : kw + WO,
            ]
        else:
            dsl = xst[:, kh : kh + R_TD, kw : kw + WO]
            msl = xst[:, R_TD + kh : R_TD + kh + R_ACT, kw : kw + WO]
        sc = wvt[:, j, t : t + 1]
        if i == 0:
            # first term writes accumulators directly
            nc.vector.tensor_scalar(
                out=od[:], in0=dsl[:, :R_DVE, :], scalar1=sc, scalar2=None,
                op0=mult,
            )
            nc.vector.tensor_scalar(
                out=ov[:, :R_PTD, :], in0=dsl[:, R_DVE:, :], scalar1=sc,
                scalar2=None, op0=mult,
            )
            nc.scalar.activation(ov[:, R_PTD:, :], msl, Copy, scale=sc)
        else:
            td = tdpool.tile([CPC, R_TD, WO], f16)
            nc.vector.tensor_scalar(
                out=td[:], in0=dsl, scalar1=sc, scalar2=None, op0=mult
            )
            nc.vector.tensor_tensor(
                out=od[:], in0=od[:], in1=td[:, :R_DVE, :], op=add
            )
            nc.gpsimd.tensor_tensor(
                out=ov[:, :R_PTD, :], in0=ov[:, :R_PTD, :],
                in1=td[:, R_DVE:, :], op=add,
            )
            tm = tmpool.tile([CPC, R_ACT, WO], f16)
            nc.scalar.activation(tm[:], msl, Copy, scale=sc)
            nc.gpsimd.tensor_tensor(
                out=ov[:, R_PTD:, :], in0=ov[:, R_PTD:, :], in1=tm[:], op=add
            )

    # ---- interleaved emission: PE chunks + vector terms + ACT evictions ----
    ti = 0
    for ci, (r0, nr) in enumerate(_CHUNKS):
        pt = ppool.tile([CPC, nr, WO], f32)
        t = 0
        for kh in range(KH):
            for kw in range(KW):
                nc.tensor.matmul(
                    pt[:],
                    lhsT=wt[:, t, :],
                    rhs=xt[:, r0 + kh : r0 + kh + nr, kw : kw + WO],
                    start=(t == 0),
                    stop=(t == KH * KW - 1),
                )
                t += 1
        for _ in range(_TERMS_BEFORE_EVICT[ci]):
            emit_term(ti)
            ti += 1
        nc.scalar.activation(op[:, r0 : r0 + nr, :], pt[:], Copy)
        if r0 + nr == OSPLIT:
            nc.sync.dma_start(out=y_d[n, :, :OSPLIT, :], in_=op[:, :OSPLIT, :])
    while ti < len(taps):
        emit_term(ti)
        ti += 1

    nc.sync.dma_start(
        out=y_d[n, :, OSPLIT:R_PE, :], in_=op[:, OSPLIT:, :]
    )
    nc.sync.dma_start(
        out=y_d[n, :, VSTART : VSTART + R_DVE, :], in_=od[:]
    )
    nc.sync.dma_start(out=y_d[n, :, VSTART + R_DVE :, :], in_=ov[:])


def _get_nc():
    if "nc" not in _NC_CACHE:
        _NC_CACHE["nc"] = _build_program()
    return _NC_CACHE["nc"]


def _make_wmats(w):
    """Per-core lhsT weight mats, shape (128, 9, 128): wm[ic, t, oc]."""
    oc = np.arange(CPC)
    mats = []
    for cid in range(N_CORES):
        ws = np.asarray(w[cid * CPC : (cid + 1) * CPC], dtype=np.float32)
        wm = np.zeros((CPC, KH * KW, CPC), dtype=np.float32)
        for icg in range(2):
            ic = (oc // 2) * 2 + icg
            wm[ic, :, oc] = ws[oc, icg].reshape(CPC, KH * KW)
        mats.append(wm.astype(np.float16))
    return mats


def _make_wvecs(w):
    """Per-core diag/cross scalar tables, shape (128, 2, 9) fp32."""
    p = np.arange(CPC)
    vecs = []
    for cid in range(N_CORES):
        ws = np.asarray(w[cid * CPC : (cid + 1) * CPC], dtype=np.float32)
        wv = np.empty((CPC, 2, KH * KW), dtype=np.float32)
        wv[:, 0, :] = ws[p, p % 2].reshape(CPC, KH * KW)
        wv[:, 1, :] = ws[p, 1 - p % 2].reshape(CPC, KH * KW)
        vecs.append(wv)
    return vecs


def _run(x, w, trace=False, **kwargs):
    nc = _get_nc()
    x = np.asarray(x)
    perm = np.arange(CPC) ^ 1
    wmats = _make_wmats(w)
    wvecs = _make_wvecs(w)
    in_maps = []
    for cid in range(N_CORES):
        xc = np.ascontiguousarray(
            x[:, cid * CPC : (cid + 1) * CPC], dtype=np.float16
        )
        xsc = np.ascontiguousarray(xc[:, perm, VSTART : VSTART + XS_ROWS, :])
        in_maps.append({"x": xc, "xs": xsc, "wm": wmats[cid], "wv": wvecs[cid]})
    res = run_bass_kernel_spmd(
        nc, in_maps, list(range(N_CORES)), trace=trace, **kwargs
    )
    y = np.concatenate(
        [res.results[i]["y"].astype(np.float32) for i in range(N_CORES)],
        axis=1,
    )
    return y, res


def kernel(x, w):
    y, _ = _run(x, w, trace=False)
    return y


# revision 15
# speedup vs baseline: 1.1315x; 1.1315x over previous
"""Grouped Conv2d (512 groups, 2->2 ch/group, 3x3 VALID) on 8 trn2 NeuronCores.

Strategy (hybrid, fp16 data path):
  - Shard the 512 groups across 8 cores: 64 groups = 128 channels per core.
    Fully independent (no collectives); batch stays whole on every core.
  - Row-split each batch's 54 output rows across engines:
      * PE rows [0, R_PE): block-diagonal 128x128 weight per 3x3 tap; 9
        accumulating fp16 matmuls per PSUM chunk (<=9 rows x 54 cols); ACT
        evicts PSUM -> fp16 SBUF (interleaved with its product ops).
      * DVE rows: per-term tensor_scalar product (4x mode) into td, then
        tensor_tensor accumulate (2x mode); 18 terms = 9 taps x {diag,
        pair-swapped}.
      * Pool rows: accumulated on gpsimd from td tail rows + ACT products.
    Per-group 2x2 channel mixing needs x[p^1] at partition p: host supplies
    a channel pair-swapped copy `xs` of the bottom x rows.
  - fp16 halves DMA bytes vs fp32 (x converted on host; y back on host).
  - Last batches run PE-heavier (TAIL) so the vector engines don't trail
    the PE after its final chunk.
"""

import sys

import numpy as np

for _p in ("/opt/trn_rl_repo",):
    if _p not in sys.path:
        sys.path.insert(0, _p)

import concourse.bacc as bacc
import concourse.bass as bass
import concourse.tile as tile
from concourse import mybir
from concourse.bass_utils import run_bass_kernel_spmd

N_CORES = 8
B, C, H, W = 16, 1024, 56, 56
KH = KW = 3
HO, WO = H - KH + 1, W - KW + 1  # 54, 54
CPC = C // N_CORES  # 128 channels (64 groups) per core

# Row split (mutable for sweeps; kernel() uses these values at build time)
CFG = {
    "R_PE": 43,   # rows on the PE (psum chunks of <=9 rows)
    "R_DVE": 9,   # rows accumulated on DVE
    "R_PTD": 0,   # rows accumulated on Pool from DVE-produced td tail
    # rest of the 54 rows: produced by ACT, added on Pool
    "TBE": (7, 5, 5, 1, 0),  # vector terms emitted before each eviction
    "EVICT": "AAAAAAA",  # eviction engine per chunk: A(CT)/D(VE)
    "WARMUP": 16,  # dummy matmuls before batch 0
    "XSWAP": False,  # load x rows 30:56 before rows 0:30
    "TAIL": (),  # per-batch R_PE overrides for the last len(TAIL) batches
}

_NC_CACHE = {}


def _chunks(r_pe):
    out, r = [], 0
    while r < r_pe:
        out.append((r, min(9, r_pe - r)))
        r += min(9, r_pe - r)
    return out


def _batch_geoms(cfg):
    """Per-batch (r_pe, r_dve, r_ptd, r_act) honoring TAIL overrides."""
    geoms = []
    tail = cfg["TAIL"]
    for n in range(B):
        r_pe = cfg["R_PE"]
        if tail and n >= B - len(tail):
            r_pe = tail[n - (B - len(tail))]
        r_v = HO - r_pe
        r_dve = min(cfg["R_DVE"], r_v)
        r_ptd = min(cfg["R_PTD"], r_v - r_dve)
        r_act = r_v - r_dve - r_ptd
        geoms.append((r_pe, r_dve, r_ptd, r_act))
    return geoms


def _build_program(cfg):
    geoms = _batch_geoms(cfg)
    vstart_min = min(g[0] for g in geoms)
    xs_rows = HO - vstart_min + KH - 1
    nc = bacc.Bacc(
        "TRN2", target_bir_lowering=False, debug=False, num_devices=N_CORES
    )
    f32 = mybir.dt.float32
    f16 = mybir.dt.float16

    x_d = nc.declare_dram_parameter("x", [B, CPC, H, W], f16, isOutput=False)
    xs_d = nc.declare_dram_parameter(
        "xs", [B, CPC, xs_rows, W], f16, isOutput=False
    )
    wm_d = nc.declare_dram_parameter(
        "wm", [CPC, KH * KW, CPC], f16, isOutput=False
    )
    wv_d = nc.declare_dram_parameter("wv", [CPC, 2, KH * KW], f32, isOutput=False)
    y_d = nc.declare_dram_parameter("y", [B, CPC, HO, WO], f16, isOutput=True)

    with tile.TileContext(nc) as tc:
        with (
            tc.tile_pool(name="wpool", bufs=1) as wpool,
            tc.tile_pool(name="xpool", bufs=3) as xpool,
            tc.tile_pool(name="xspool", bufs=3) as xspool,
            tc.tile_pool(name="oppool", bufs=3) as oppool,
            tc.tile_pool(name="odpool", bufs=3) as odpool,
            tc.tile_pool(name="ovpool", bufs=3) as ovpool,
            tc.tile_pool(name="tdpool", bufs=4) as tdpool,
            tc.tile_pool(name="tmpool", bufs=4) as tmpool,
            tc.tile_pool(name="psum", bufs=7, space="PSUM") as ppool,
            tc.tile_pool(name="scratch", bufs=1, space="PSUM") as spool,
        ):
            wt = wpool.tile([CPC, KH * KW, CPC], f16)
            # tap-0 weights land first so PE warmup starts ASAP
            nc.sync.dma_start(out=wt[:, 0:1, :], in_=wm_d[:, 0:1, :])
            wvt = wpool.tile([CPC, 2, KH * KW], f32)

            # The fused matmul (LDW+MM) supports only ONE semaphore wait;
            # these sync matmuls absorb DMA waits so real matmuls only
            # depend on PE program order.
            scr = spool.tile([CPC, 512], f32)
            nc.tensor.matmul(
                scr[:, :2], lhsT=wt[:, 0, :], rhs=wt[:, 0, :2],
                start=True, stop=True,
            )
            # Dummy matmuls keep PE busy during the initial x DMA fill so
            # the HAM clock gate ramps to 2.4 GHz before real work arrives.
            for _ in range(cfg["WARMUP"]):
                nc.tensor.matmul(
                    scr[:, :128], lhsT=wt[:, 0, :], rhs=wt[:, 0, :],
                    start=True, stop=True,
                )

            def load_rest():
                # bulk weights stream in behind batch 0's first x rows
                nc.sync.dma_start(out=wt[:, 1:, :], in_=wm_d[:, 1:, :])
                nc.sync.dma_start(out=wvt[:], in_=wv_d[:])
                nc.tensor.matmul(
                    scr[:, :2], lhsT=wt[:, 0, :], rhs=wt[:, 8, :2],
                    start=True, stop=True,
                )

            pools = (xpool, xspool, oppool, odpool, ovpool, tdpool, tmpool, ppool)
            for n in range(B):
                _emit_batch(
                    nc, pools, geoms[n], vstart_min, cfg,
                    x_d, xs_d, y_d, wt, wvt, scr, n,
                    load_rest if n == 0 else None,
                )
    nc.compile()
    return nc


def _emit_batch(
    nc, pools, geom, vstart_min, cfg, x_d, xs_d, y_d, wt, wvt, scr, n,
    load_rest=None,
):
    (xpool, xspool, oppool, odpool, ovpool, tdpool, tmpool, ppool) = pools
    (r_pe, r_dve, r_ptd, r_act) = geom
    chunks = _chunks(r_pe)
    r_td = r_dve + r_ptd
    xs_rows = HO - vstart_min + KH - 1
    f32 = mybir.dt.float32
    f16 = mybir.dt.float16
    HSPLIT = 30
    OSPLIT = 27
    vstart = r_pe
    xso = vstart - vstart_min  # row offset of this batch's region within xst
    Copy = mybir.ActivationFunctionType.Copy
    add = mybir.AluOpType.add
    mult = mybir.AluOpType.mult

    xt = xpool.tile([CPC, H, W], f16)
    if load_rest is not None:
        # batch 0: first chunk's rows land first; bulk weights follow them
        halves = [(slice(0, 11), 0), (slice(11, HSPLIT), 11)]
        for sl, _ in halves:
            nc.sync.dma_start(out=xt[:, sl, :], in_=x_d[n, :, sl, :])
        load_rest()
        nc.sync.dma_start(out=xt[:, HSPLIT:, :], in_=x_d[n, :, HSPLIT:, :])
        halves.append((slice(HSPLIT, H), H - 1))
    else:
        halves = [(slice(0, HSPLIT), 0), (slice(HSPLIT, H), H - 1)]
        if cfg["XSWAP"]:
            halves.reverse()
        for sl, _ in halves:
            nc.sync.dma_start(out=xt[:, sl, :], in_=x_d[n, :, sl, :])
    xst = xspool.tile([CPC, xs_rows, W], f16)
    nc.sync.dma_start(out=xst[:], in_=xs_d[n])

    # absorb the x-DMA semaphores ahead of the real matmuls
    for _, row in halves:
        nc.tensor.matmul(
            scr[:, :2], lhsT=wt[:, 0, :], rhs=xt[:, row, :2],
            start=True, stop=True,
        )

    op = oppool.tile([CPC, r_pe, WO], f16)   # PE-region rows
    od = None
    if r_dve > 0:
        od = odpool.tile([CPC, r_dve, WO], f16)  # DVE-accumulated rows
    ov = None
    if r_ptd + r_act > 0:
        ov = ovpool.tile([CPC, r_ptd + r_act, WO], f16)  # Pool rows

    taps = [(kh, kw, j) for kh in range(KH) for kw in range(KW) for j in range(2)]

    def emit_term(i):
        kh, kw, j = taps[i]
        t = kh * KW + kw
        if j == 0:
            dsl = xt[:, vstart + kh : vstart + kh + r_td, kw : kw + WO]
            msl = xt[
                :,
                vstart + r_td + kh : vstart + r_td + kh + r_act,
                kw : kw + WO,
            ]
        else:
            dsl = xst[:, xso + kh : xso + kh + r_td, kw : kw + WO]
            msl = xst[
                :, xso + r_td + kh : xso + r_td + kh + r_act, kw : kw + WO
            ]
        sc = wvt[:, j, t : t + 1]
        if i == 0:
            # first term writes accumulators directly
            if r_dve:
                nc.vector.tensor_scalar(
                    out=od[:], in0=dsl[:, :r_dve, :], scalar1=sc,
                    scalar2=None, op0=mult,
                )
            if r_ptd:
                nc.vector.tensor_scalar(
                    out=ov[:, :r_ptd, :], in0=dsl[:, r_dve:, :], scalar1=sc,
                    scalar2=None, op0=mult,
                )
            if r_act:
                nc.scalar.activation(ov[:, r_ptd:, :], msl, Copy, scale=sc)
        else:
            if r_td:
                td = tdpool.tile([CPC, r_td, WO], f16)
                nc.vector.tensor_scalar(
                    out=td[:], in0=dsl, scalar1=sc, scalar2=None, op0=mult
                )
                if r_dve:
                    nc.vector.tensor_tensor(
                        out=od[:], in0=od[:], in1=td[:, :r_dve, :], op=add
                    )
            if r_ptd:
                nc.gpsimd.tensor_tensor(
                    out=ov[:, :r_ptd, :], in0=ov[:, :r_ptd, :],
                    in1=td[:, r_dve:, :], op=add,
                )
            if r_act:
                tm = tmpool.tile([CPC, r_act, WO], f16)
                nc.scalar.activation(tm[:], msl, Copy, scale=sc)
                nc.gpsimd.tensor_tensor(
                    out=ov[:, r_ptd:, :], in0=ov[:, r_ptd:, :], in1=tm[:],
                    op=add,
                )

    # ---- interleaved emission: PE chunks + vector terms + evictions ----
    tbe = list(cfg["TBE"])[: len(chunks)]
    tbe += [0] * (len(chunks) - len(tbe))
    ti = 0
    n_terms = len(taps) if (r_td or r_act) else 0
    for ci, (r0, nr) in enumerate(chunks):
        pt = ppool.tile([CPC, nr, WO], f32)
        t = 0
        for kh in range(KH):
            for kw in range(KW):
                nc.tensor.matmul(
                    pt[:],
                    lhsT=wt[:, t, :],
                    rhs=xt[:, r0 + kh : r0 + kh + nr, kw : kw + WO],
                    start=(t == 0),
                    stop=(t == KH * KW - 1),
                )
                t += 1
        for _ in range(tbe[ci]):
            if ti < n_terms:
                emit_term(ti)
                ti += 1
        if cfg["EVICT"][ci] == "A":
            nc.scalar.activation(op[:, r0 : r0 + nr, :], pt[:], Copy)
        else:
            nc.vector.tensor_copy(op[:, r0 : r0 + nr, :], pt[:])
        if r0 + nr == OSPLIT:
            nc.sync.dma_start(out=y_d[n, :, :OSPLIT, :], in_=op[:, :OSPLIT, :])
    while ti < n_terms:
        emit_term(ti)
        ti += 1

    nc.sync.dma_start(out=y_d[n, :, OSPLIT:r_pe, :], in_=op[:, OSPLIT:, :])
    if od is not None:
        nc.sync.dma_start(out=y_d[n, :, vstart : vstart + r_dve, :], in_=od[:])
    if ov is not None:
        nc.sync.dma_start(out=y_d[n, :, vstart + r_dve :, :], in_=ov[:])


def _get_nc():
    key = repr(sorted(CFG.items()))
    if key not in _NC_CACHE:
        _NC_CACHE[key] = _build_program(CFG)
    return _NC_CACHE[key]


def _make_wmats(w):
    """Per-core lhsT weight mats, shape (128, 9, 128): wm[ic, t, oc]."""
    oc = np.arange(CPC)
    mats = []
    for cid in range(N_CORES):
        ws = np.asarray(w[cid * CPC : (cid + 1) * CPC], dtype=np.float32)
        wm = np.zeros((CPC, KH * KW, CPC), dtype=np.float32)
        for icg in range(2):
            ic = (oc // 2) * 2 + icg
            wm[ic, :, oc] = ws[oc, icg].reshape(CPC, KH * KW)
        mats.append(wm.astype(np.float16))
    return mats


def _make_wvecs(w):
    """Per-core diag/cross scalar tables, shape (128, 2, 9) fp32."""
    p = np.arange(CPC)
    vecs = []
    for cid in range(N_CORES):
        ws = np.asarray(w[cid * CPC : (cid + 1) * CPC], dtype=np.float32)
        wv = np.empty((CPC, 2, KH * KW), dtype=np.float32)
        wv[:, 0, :] = ws[p, p % 2].reshape(CPC, KH * KW)
        wv[:, 1, :] = ws[p, 1 - p % 2].reshape(CPC, KH * KW)
        vecs.append(wv)
    return vecs


def _run(x, w, trace=False, **kwargs):
    nc = _get_nc()
    geoms = _batch_geoms(CFG)
    vstart_min = min(g[0] for g in geoms)
    xs_rows = HO - vstart_min + KH - 1
    x = np.asarray(x)
    perm = np.arange(CPC) ^ 1
    wmats = _make_wmats(w)
    wvecs = _make_wvecs(w)
    in_maps = []
    for cid in range(N_CORES):
        xc = np.ascontiguousarray(
            x[:, cid * CPC : (cid + 1) * CPC], dtype=np.float16
        )
        xsc = np.ascontiguousarray(
            xc[:, perm, vstart_min : vstart_min + xs_rows, :]
        )
        in_maps.append({"x": xc, "xs": xsc, "wm": wmats[cid], "wv": wvecs[cid]})
    res = run_bass_kernel_spmd(
        nc, in_maps, list(range(N_CORES)), trace=trace, **kwargs
    )
    y = np.concatenate(
        [res.results[i]["y"].astype(np.float32) for i in range(N_CORES)],
        axis=1,
    )
    return y, res


def kernel(x, w):
    y, _ = _run(x, w, trace=False)
    return y


# revision 30
# speedup vs baseline: 1.1712x; 1.0351x over previous
"""Grouped Conv2d (512 groups, 2->2 ch/group, 3x3 VALID) on 8 trn2 NeuronCores.

Strategy (hybrid, fp16 data path):
  - Shard the 512 groups across 8 cores: 64 groups = 128 channels per core.
    Fully independent (no collectives); batch stays whole on every core.
  - Row-split each batch's 54 output rows across engines:
      * PE rows [0, R_PE): block-diagonal 128x128 weight per 3x3 tap; 9
        accumulating fp16 matmuls per PSUM chunk (<=9 rows x 54 cols); ACT
        evicts PSUM -> fp16 SBUF (interleaved with its product ops).
      * DVE rows: per-term tensor_scalar product (4x mode) + tensor_tensor
        accumulate (2x mode); 18 terms = 9 taps x {diag, pair-swapped}.
      * ACT-product rows: ACT per-partition-scale products, gpsimd adds.
    Per-group 2x2 channel mixing needs x[p^1] at partition p: host supplies
    a channel pair-swapped copy `xs` of the bottom x rows.
  - K batches are fused per vector-engine op (4D access patterns) to
    amortize per-op fixed costs; the PE still works chunk-by-chunk per
    batch. The next super-batch's loads are emitted before this one's
    final stores so prefetch is never queued behind them.
  - fp16 halves DMA bytes vs fp32 (x converted on host; y back on host).
"""

import sys

import numpy as np

for _p in ("/opt/trn_rl_repo",):
    if _p not in sys.path:
        sys.path.insert(0, _p)

import concourse.bacc as bacc
import concourse.bass as bass
import concourse.tile as tile
from concourse import mybir
from concourse.bass_utils import run_bass_kernel_spmd

N_CORES = 8
B, C, H, W = 16, 1024, 56, 56
KH = KW = 3
HO, WO = H - KH + 1, W - KW + 1  # 54, 54
CPC = C // N_CORES  # 128 channels (64 groups) per core

# Row split (mutable for sweeps; kernel() uses these values at build time)
CFG = {
    "K": 2,       # batches fused per vector-engine op
    "R_PE": 42,   # rows on the PE (psum chunks of <=9 rows)
    "R_DVE": 9,   # rows accumulated on DVE
    # rest of the 54 rows: produced by ACT, added on Pool
    "POOL_TT": 0,  # DVE-row add-terms offloaded to gpsimd
    "WARMUP": 16,  # dummy matmuls before batch 0
}

_NC_CACHE = {}

HSPLIT = 30
OSPLIT = 27


def _chunks(r_pe):
    out, r = [], 0
    while r < r_pe:
        out.append((r, min(9, r_pe - r)))
        r += min(9, r_pe - r)
    return out


def _build_program(cfg):
    k = cfg["K"]
    assert B % k == 0
    r_pe = cfg["R_PE"]
    r_dve = cfg["R_DVE"]
    r_act = HO - r_pe - r_dve
    assert r_dve > 0 and r_act >= 0
    vstart = r_pe
    xs_rows = HO - r_pe + KH - 1
    chunks = _chunks(r_pe)

    nc = bacc.Bacc(
        "TRN2", target_bir_lowering=False, debug=False, num_devices=N_CORES
    )
    f32 = mybir.dt.float32
    f16 = mybir.dt.float16
    Copy = mybir.ActivationFunctionType.Copy
    add = mybir.AluOpType.add
    mult = mybir.AluOpType.mult

    x_d = nc.declare_dram_parameter("x", [B, CPC, H, W], f16, isOutput=False)
    xs_d = nc.declare_dram_parameter(
        "xs", [B, CPC, xs_rows, W], f16, isOutput=False
    )
    wm_d = nc.declare_dram_parameter(
        "wm", [CPC, KH * KW, CPC], f16, isOutput=False
    )
    wv_d = nc.declare_dram_parameter("wv", [CPC, 2, KH * KW], f32, isOutput=False)
    y_d = nc.declare_dram_parameter("y", [B, CPC, HO, WO], f16, isOutput=True)

    with tile.TileContext(nc) as tc:
        with (
            tc.tile_pool(name="wpool", bufs=1) as wpool,
            tc.tile_pool(name="xpool", bufs=2) as xpool,
            tc.tile_pool(name="xspool", bufs=2) as xspool,
            tc.tile_pool(name="oppool", bufs=2) as oppool,
            tc.tile_pool(name="odpool", bufs=2) as odpool,
            tc.tile_pool(name="ovpool", bufs=2) as ovpool,
            tc.tile_pool(name="tdpool", bufs=3) as tdpool,
            tc.tile_pool(name="tmpool", bufs=3) as tmpool,
            tc.tile_pool(name="psum", bufs=7, space="PSUM") as ppool,
            tc.tile_pool(name="scratch", bufs=1, space="PSUM") as spool,
        ):
            wt = wpool.tile([CPC, KH * KW, CPC], f16)
            # tap-0 weights land first so PE warmup starts ASAP
            nc.sync.dma_start(out=wt[:, 0:1, :], in_=wm_d[:, 0:1, :])
            wvt = wpool.tile([CPC, 2, KH * KW], f32)

            # The fused matmul (LDW+MM) supports only ONE semaphore wait;
            # sync matmuls absorb DMA waits so real matmuls only depend on
            # PE program order.
            scr = spool.tile([CPC, 512], f32)
            nc.tensor.matmul(
                scr[:, :2], lhsT=wt[:, 0, :], rhs=wt[:, 0, :2],
                start=True, stop=True,
            )
            # Dummy matmuls keep PE busy during the initial x DMA fill so
            # the HAM clock gate ramps to 2.4 GHz before real work arrives.
            for _ in range(cfg["WARMUP"]):
                nc.tensor.matmul(
                    scr[:, :128], lhsT=wt[:, 0, :], rhs=wt[:, 0, :],
                    start=True, stop=True,
                )

            taps = [
                (kh, kw, j)
                for kh in range(KH) for kw in range(KW) for j in range(2)
            ]
            n_supers = B // k

            def emit_loads(s):
                n0 = s * k
                xt = xpool.tile([CPC, k, H, W], f16)
                if s == 0:
                    # batch 0's first chunk rows land first; bulk weights
                    # stream right behind them
                    nc.sync.dma_start(
                        out=xt[:, 0, :11, :], in_=x_d[n0, :, :11, :]
                    )
                    nc.sync.dma_start(out=wt[:, 1:, :], in_=wm_d[:, 1:, :])
                    nc.sync.dma_start(
                        out=xt[:, 0, 11:HSPLIT, :], in_=x_d[n0, :, 11:HSPLIT, :]
                    )
                    nc.sync.dma_start(out=wvt[:], in_=wv_d[:])
                    nc.tensor.matmul(
                        scr[:, :2], lhsT=wt[:, 0, :], rhs=wt[:, 8, :2],
                        start=True, stop=True,
                    )
                    nc.sync.dma_start(
                        out=xt[:, 0, HSPLIT:, :], in_=x_d[n0, :, HSPLIT:, :]
                    )
                    rest = range(1, k)
                else:
                    rest = range(k)
                for b in rest:
                    nc.sync.dma_start(
                        out=xt[:, b, :HSPLIT, :], in_=x_d[n0 + b, :, :HSPLIT, :]
                    )
                    nc.sync.dma_start(
                        out=xt[:, b, HSPLIT:, :], in_=x_d[n0 + b, :, HSPLIT:, :]
                    )
                xst = xspool.tile([CPC, k, xs_rows, W], f16)
                for b in range(k):
                    nc.sync.dma_start(out=xst[:, b], in_=xs_d[n0 + b])
                return xt, xst

            def emit_super(s, tiles, next_tiles_loader):
                n0 = s * k
                xt, xst = tiles
                # absorb x-DMA semaphores ahead of the real matmuls
                for b in range(k):
                    for row in (0, H - 1):
                        nc.tensor.matmul(
                            scr[:, :2], lhsT=wt[:, 0, :],
                            rhs=xt[:, b, row, :2], start=True, stop=True,
                        )

                # prefetch next super's inputs ahead of our y stores so the
                # loads never queue behind them on the DMA path
                nxt = next_tiles_loader() if next_tiles_loader else None

                ops = [
                    oppool.tile([CPC, r_pe, WO], f16, name=f"op{b}")
                    for b in range(k)
                ]
                od = odpool.tile([CPC, k, r_dve, WO], f16)
                ov = None
                if r_act:
                    ov = ovpool.tile([CPC, k, r_act, WO], f16)

                def emit_term(i):
                    kh, kw, j = taps[i]
                    t = kh * KW + kw
                    if j == 0:
                        dsl = xt[
                            :, :, vstart + kh : vstart + kh + r_dve,
                            kw : kw + WO,
                        ]
                        msl = xt[
                            :, :,
                            vstart + r_dve + kh : vstart + r_dve + kh + r_act,
                            kw : kw + WO,
                        ]
                    else:
                        dsl = xst[:, :, kh : kh + r_dve, kw : kw + WO]
                        msl = xst[
                            :, :, r_dve + kh : r_dve + kh + r_act, kw : kw + WO
                        ]
                    sc = wvt[:, j, t : t + 1]
                    if i == 0:
                        nc.vector.tensor_scalar(
                            out=od[:], in0=dsl, scalar1=sc, scalar2=None,
                            op0=mult,
                        )
                        if r_act:
                            nc.scalar.activation(ov[:], msl, Copy, scale=sc)
                    else:
                        td = tdpool.tile([CPC, k, r_dve, WO], f16)
                        nc.vector.tensor_scalar(
                            out=td[:], in0=dsl, scalar1=sc, scalar2=None,
                            op0=mult,
                        )
                        if i <= cfg["POOL_TT"]:
                            nc.gpsimd.tensor_tensor(
                                out=od[:], in0=od[:], in1=td[:], op=add
                            )
                        else:
                            nc.vector.tensor_tensor(
                                out=od[:], in0=od[:], in1=td[:], op=add
                            )
                        if r_act:
                            tm = tmpool.tile([CPC, k, r_act, WO], f16)
                            nc.scalar.activation(tm[:], msl, Copy, scale=sc)
                            nc.gpsimd.tensor_tensor(
                                out=ov[:], in0=ov[:], in1=tm[:], op=add
                            )

                ti = 0
                n_terms = len(taps)
                last_piece = OSPLIT
                for b in range(k):
                    op = ops[b]
                    for ci, (r0, nr) in enumerate(chunks):
                        pt = ppool.tile([CPC, nr, WO], f32)
                        t = 0
                        for kh in range(KH):
                            for kw in range(KW):
                                nc.tensor.matmul(
                                    pt[:],
                                    lhsT=wt[:, t, :],
                                    rhs=xt[
                                        :, b, r0 + kh : r0 + kh + nr,
                                        kw : kw + WO,
                                    ],
                                    start=(t == 0),
                                    stop=(t == KH * KW - 1),
                                )
                                t += 1
                        if ti < n_terms:
                            emit_term(ti)
                            ti += 1
                        nc.scalar.activation(op[:, r0 : r0 + nr, :], pt[:], Copy)
                        if r0 + nr == OSPLIT:
                            nc.sync.dma_start(
                                out=y_d[n0 + b, :, :OSPLIT, :],
                                in_=op[:, :OSPLIT, :],
                            )
                        elif ci == len(chunks) - 2:
                            nc.sync.dma_start(
                                out=y_d[n0 + b, :, OSPLIT : r0 + nr, :],
                                in_=op[:, OSPLIT : r0 + nr, :],
                            )
                            last_piece = r0 + nr
                while ti < n_terms:
                    emit_term(ti)
                    ti += 1

                # vector-region stores first: their producers run ahead of
                # the PE, so these must not queue behind the last yp piece
                for b in range(k):
                    nc.sync.dma_start(
                        out=y_d[n0 + b, :, vstart : vstart + r_dve, :],
                        in_=od[:, b],
                    )
                if ov is not None:
                    for b in range(k):
                        nc.sync.dma_start(
                            out=y_d[n0 + b, :, vstart + r_dve :, :],
                            in_=ov[:, b],
                        )
                for b in range(k):
                    nc.sync.dma_start(
                        out=y_d[n0 + b, :, last_piece:r_pe, :],
                        in_=ops[b][:, last_piece:, :],
                    )
                return nxt

            tiles = emit_loads(0)
            for s in range(n_supers):
                loader = (
                    (lambda s1=s + 1: emit_loads(s1))
                    if s + 1 < n_supers else None
                )
                tiles = emit_super(s, tiles, loader)
    nc.compile()
    return nc


def _get_nc():
    key = repr(sorted(CFG.items()))
    if key not in _NC_CACHE:
        _NC_CACHE[key] = _build_program(CFG)
    return _NC_CACHE[key]


def _make_wmats(w):
    """Per-core lhsT weight mats, shape (128, 9, 128): wm[ic, t, oc]."""
    oc = np.arange(CPC)
    mats = []
    for cid in range(N_CORES):
        ws = np.asarray(w[cid * CPC : (cid + 1) * CPC], dtype=np.float32)
        wm = np.zeros((CPC, KH * KW, CPC), dtype=np.float32)
        for icg in range(2):
            ic = (oc // 2) * 2 + icg
            wm[ic, :, oc] = ws[oc, icg].reshape(CPC, KH * KW)
        mats.append(wm.astype(np.float16))
    return mats


def _make_wvecs(w):
    """Per-core diag/cross scalar tables, shape (128, 2, 9) fp32."""
    p = np.arange(CPC)
    vecs = []
    for cid in range(N_CORES):
        ws = np.asarray(w[cid * CPC : (cid + 1) * CPC], dtype=np.float32)
        wv = np.empty((CPC, 2, KH * KW), dtype=np.float32)
        wv[:, 0, :] = ws[p, p % 2].reshape(CPC, KH * KW)
        wv[:, 1, :] = ws[p, 1 - p % 2].reshape(CPC, KH * KW)
        vecs.append(wv)
    return vecs


def _run(x, w, trace=False, **kwargs):
    nc = _get_nc()
    vstart = CFG["R_PE"]
    xs_rows = HO - vstart + KH - 1
    x = np.asarray(x)
    perm = np.arange(CPC) ^ 1
    wmats = _make_wmats(w)
    wvecs = _make_wvecs(w)
    in_maps = []
    for cid in range(N_CORES):
        xc = np.ascontiguousarray(
            x[:, cid * CPC : (cid + 1) * CPC], dtype=np.float16
        )
        xsc = np.ascontiguousarray(xc[:, perm, vstart : vstart + xs_rows, :])
        in_maps.append({"x": xc, "xs": xsc, "wm": wmats[cid], "wv": wvecs[cid]})
    res = run_bass_kernel_spmd(
        nc, in_maps, list(range(N_CORES)), trace=trace, **kwargs
    )
    y = np.concatenate(
        [res.results[i]["y"].astype(np.float32) for i in range(N_CORES)],
        axis=1,
    )
    return y, res


def kernel(x, w):
    y, _ = _run(x, w, trace=False)
    return y
